# revision 1
# baseline (speedup 1.0000x reference)
"""Dense lockstep AffineNet kernel v5: stage-2 via GPSIMD partition-reduce.

Same math as before, but DVE/ACT work is done in [128, 4096] mega-tiles and
mask-multiplies are batched 4 batches at a time through a 4-bank PSUM tile,
cutting total instruction count (the dominant cost on this stack).
"""
import numpy as np

B, C, H, W = 8, 8, 256, 256
HW = H * W
P = 128
CH = 512            # matmul N (PSUM f32 bank limit)
MS = 2048           # mega-tile px (DVE/ACT granularity)
NMEGA = HW // MS    # 16
SUB = MS // CH      # 8 sub-chunks per mega

_cache = {}


def _build_l1(repeat=1):
    import concourse.bacc as bacc
    import concourse.mybir as mybir
    import concourse.tile as tile
    nc = bacc.Bacc("TRN2", target_bir_lowering=False, debug=False)
    xs = nc.dram_tensor("xs", [C, HW], mybir.dt.float32, kind="ExternalInput")
    xbar = nc.dram_tensor("xbar", [1, HW], mybir.dt.float32, kind="ExternalOutput")
    xs_t = xs.ap().rearrange("c (p s) -> c p s", p=P)
    xb_t = xbar.ap().rearrange("one (p s) -> (one p) s", p=P)
    with tile.TileContext(nc) as tc:
        with tc.tile_pool(name="pl", bufs=2) as pool:
            for _ in range(repeat):
                ts = [pool.tile([P, HW // P], mybir.dt.float32, tag=f"i{c}", name=f"i{c}") for c in range(C)]
                for c in range(C):
                    nc.sync.dma_start(ts[c][:], xs_t[c])
                acc = pool.tile([P, HW // P], mybir.dt.float32, tag="acc", name="acc")
                nc.vector.tensor_add(acc[:], ts[0][:], ts[1][:])
                for c in range(2, C):
                    nc.vector.tensor_add(acc[:], acc[:], ts[c][:])
                nc.vector.tensor_scalar_mul(acc[:], acc[:], 1.0 / C)
                nc.sync.dma_start(xb_t, acc[:])
    nc.compile()
    return nc


def _build_l2(repeat=1):
    import concourse.bacc as bacc
    import concourse.bass as bass
    import concourse.mybir as mybir
    import concourse.tile as tile
    f32 = mybir.dt.float32
    Alu = mybir.AluOpType
    Act = mybir.ActivationFunctionType

    nc = bacc.Bacc("TRN2", target_bir_lowering=False, debug=False)
    xb = nc.dram_tensor("xb", [B, H, W], f32, kind="ExternalInput")
    tht = nc.dram_tensor("tht", [P, 8], f32, kind="ExternalInput")
    ubd = nc.dram_tensor("ub", [P, MS], f32, kind="ExternalInput")   # t & 255
    vbd = nc.dram_tensor("vb", [P, MS], f32, kind="ExternalInput")   # t >> 8
    pio = nc.dram_tensor("pio", [P, 2], f32, kind="ExternalInput")   # [p, 1.0]
    seld = nc.dram_tensor("sel", [P, 64], f32, kind="ExternalInput")
    res = nc.dram_tensor("res", [B, HW], f32, kind="ExternalOutput")

    with tile.TileContext(nc) as tc:
        with (
            tc.tile_pool(name="const", bufs=1) as cpool,
            tc.tile_pool(name="wk", bufs=1) as wpool,
            tc.tile_pool(name="mk", bufs=1) as mpool,
            tc.tile_pool(name="ps", bufs=1, space="PSUM") as ppool,
            tc.tile_pool(name="po", bufs=2, space="PSUM") as opool,
            tc.tile_pool(name="ro", bufs=1) as rpool,
        ):
            th = cpool.tile([P, 8], f32, name="th")
            ub = cpool.tile([P, MS], f32, name="ub")
            vb = cpool.tile([P, MS], f32, name="vb")
            pi = cpool.tile([P, 2], f32, name="pi")
            sel = cpool.tile([P, 64], f32, name="sel")
            nc.sync.dma_start(th[:], tht[:])
            nc.sync.dma_start(ub[:], ubd[:])
            nc.sync.dma_start(vb[:], vbd[:])
            nc.sync.dma_start(pi[:], pio[:])
            nc.sync.dma_start(sel[:], seld[:])
            xbt = []
            for b in range(B):
                row = []
                for rh in range(2):
                    row2 = []
                    for chh in range(2):
                        t = cpool.tile([P, P], f32, name=f"xb{b}{rh}{chh}")
                        src = bass.AP(xb.ap().tensor, b * HW + rh * 128 * W + chh * 128,
                                      [[W, 128], [1, 128]])
                        nc.sync.dma_start(t[:], src)
                        row2.append(t)
                    row.append(row2)
                xbt.append(row)

            cxy = cpool.tile([P, 2], f32, name="cxy")
            nc.vector.tensor_scalar_add(cxy[:, 0:1], th[:, 2:3], 1.0)
            nc.vector.tensor_tensor(cxy[:, 0:1], cxy[:, 0:1], th[:, 0:1], op=Alu.subtract)
            nc.vector.tensor_tensor(cxy[:, 0:1], cxy[:, 0:1], th[:, 1:2], op=Alu.subtract)
            nc.vector.tensor_scalar_mul(cxy[:, 0:1], cxy[:, 0:1], 127.5)
            nc.vector.tensor_scalar_add(cxy[:, 1:2], th[:, 5:6], 1.0)
            nc.vector.tensor_tensor(cxy[:, 1:2], cxy[:, 1:2], th[:, 3:4], op=Alu.subtract)
            nc.vector.tensor_tensor(cxy[:, 1:2], cxy[:, 1:2], th[:, 4:5], op=Alu.subtract)
            nc.vector.tensor_scalar_mul(cxy[:, 1:2], cxy[:, 1:2], 127.5)

            for _ in range(repeat):
                for mg in range(NMEGA):
                    xp = wpool.tile([P, MS], f32, tag="xp", name="xp")
                    yp = wpool.tile([P, MS], f32, tag="yp", name="yp")
                    t0 = wpool.tile([P, MS], f32, tag="t0", name="t0")
                    nc.vector.tensor_scalar(xp[:], ub[:], th[:, 0:1], None, op0=Alu.mult)
                    nc.vector.tensor_scalar_add(t0[:], vb[:], float((MS // 256) * mg))
                    nc.vector.tensor_scalar(yp[:], t0[:], th[:, 4:5], None, op0=Alu.mult)
                    nc.vector.tensor_scalar(t0[:], t0[:], th[:, 1:2], None, op0=Alu.mult)
                    nc.vector.tensor_add(xp[:], xp[:], t0[:])
                    nc.vector.tensor_scalar(xp[:], xp[:], cxy[:, 0:1], None, op0=Alu.add)
                    nc.vector.tensor_scalar(t0[:], ub[:], th[:, 3:4], None, op0=Alu.mult)
                    nc.vector.tensor_add(yp[:], yp[:], t0[:])
                    nc.vector.tensor_scalar(yp[:], yp[:], cxy[:, 1:2], None, op0=Alu.add)

                    def hat(pos, half, tag):
                        h = wpool.tile([P, MS], f32, tag=tag, name=tag)
                        nc.vector.tensor_scalar(h[:], pos[:], pi[:, 0:1],
                                                (-128.0 if half else None),
                                                op0=Alu.subtract,
                                                **({"op1": Alu.add} if half else {}))
                        nc.scalar.activation(h[:], h[:], Act.Abs)
                        nc.scalar.activation(h[:], h[:], Act.Relu, bias=1.0, scale=-1.0)
                        return h

                    wy = [hat(yp, hh, f"wy{hh}") for hh in range(2)]
                    wx = [hat(xp, hh, f"wx{hh}") for hh in range(2)]

                    for ck2 in range(SUB):
                        sl = slice(ck2 * CH, (ck2 + 1) * CH)
                        rr = rpool.tile([1, 16 * CH], f32, tag="rr", name="rr")
                        mkc = mpool.tile([P, 16 * CH], f32, tag="mkc", name="mkc")
                        for chh in range(2):
                            for bg in range(2):
                                vi4 = ppool.tile([P, 4 * CH], f32, tag="vi4", name="vi4")
                                for i in range(4):
                                    b = bg * 4 + i
                                    vs = vi4[:, i * CH:(i + 1) * CH]
                                    nc.tensor.matmul(vs, xbt[b][0][chh][:], wy[0][:, sl],
                                                     start=True, stop=False)
                                    nc.tensor.matmul(vs, xbt[b][1][chh][:], wy[1][:, sl],
                                                     start=False, stop=True)
                                wxv = bass.AP(wx[chh][:].tensor, wx[chh][:].offset + ck2 * CH,
                                              [wx[chh][:].ap[0], [0, 4], [1, CH]])
                                off = (chh * 8 + bg * 4) * CH
                                nc.vector.tensor_tensor(
                                    mkc[:, off:off + 4 * CH].rearrange(
                                        "p (i e) -> p i e", i=4),
                                    vi4[:].rearrange("p (i e) -> p i e", i=4),
                                    wxv, op=Alu.mult)
                        nc.gpsimd.tensor_reduce(rr[:], mkc[:],
                                                axis=mybir.AxisListType.C, op=Alu.add)
                        # rr = [chh0: 8b x CH | chh1: 8b x CH]; sum the halves
                        nc.vector.tensor_add(rr[:, :8 * CH], rr[:, :8 * CH],
                                             rr[:, 8 * CH:])
                        dst = bass.AP(res.ap().tensor, mg * MS + ck2 * CH,
                                      [[1, 1], [HW, 8], [1, CH]])
                        nc.sync.dma_start(dst, rr[:, :8 * CH].rearrange(
                            "one (b e) -> one b e", b=8))
    nc.compile()
    return nc


def _consts():
    t = np.arange(MS, dtype=np.int64)
    ub = np.broadcast_to((t & 255).astype(np.float32), (P, MS)).copy()
    vb = np.broadcast_to((t >> 8).astype(np.float32), (P, MS)).copy()
    pio = np.stack([np.arange(P, dtype=np.float32), np.ones(P, np.float32)], axis=1)
    sel = np.zeros((P, 64), np.float32)
    for b in range(8):
        sel[:, b * 8 + b] = 1.0
    return ub, vb, np.ascontiguousarray(pio), sel


def kernel(x, theta):
    from concourse import bass_utils
    x = np.ascontiguousarray(x, dtype=np.float32)
    theta = np.ascontiguousarray(theta, dtype=np.float32)
    if "l1" not in _cache:
        _cache["l1"] = _build_l1()
        _cache["l2"] = _build_l2()
    l1, l2 = _cache["l1"], _cache["l2"]
    cores = list(range(8))
    in1 = [{"xs": np.ascontiguousarray(x[b].reshape(C, HW))} for b in range(B)]
    r1 = bass_utils.run_bass_kernel_spmd(l1, in1, core_ids=cores)
    xbarAll = np.ascontiguousarray(
        np.stack([r1.results[b]["xbar"].reshape(H, W) for b in range(B)]))
    ub, vb, pio, sel = _consts()
    in2 = []
    for o in range(B):
        t = np.zeros(8, np.float32)
        t[:6] = theta[o].reshape(6)
        in2.append({"xb": xbarAll, "tht": np.ascontiguousarray(np.broadcast_to(t, (P, 8))),
                    "ub": ub, "vb": vb, "pio": pio, "sel": sel})
    r2 = bass_utils.run_bass_kernel_spmd(l2, in2, core_ids=cores)
    out = np.empty((B, B, H, W), np.float32)
    for o in range(B):
        out[:, o] = r2.results[o]["res"].reshape(B, H, W)
    return out



# revision 2
# speedup vs baseline: 1.0631x; 1.0631x over previous
"""AffineNet kernel v4: ap_gather 4-tap bilinear, instruction-count optimized.

Same algorithm as v3 (see kernel_v3 docstring) with:
- index pipeline computed once for all 4 rounds in wrapped layout [P, 512]
  using integer host maps HJW4/WCI4, bit-identical to the weights pipeline's
  HJD4/Wb expressions (same op structure and constants)
- theta-only precomputes hoisted out of the repeat loop
- s=1 calls combine only lane 0 (lane 1 weight is identically zero)
- L1 (mean over channels) in 4 instructions via a 3D-strided load + X-reduce
"""
import numpy as np

B, C, H, W = 8, 8, 256, 256
HW = H * W
P = 128
NI = 2048
NR = 4
RST = 130
NE = 131 * RST + 2  # 17032
MAGIC = 12582912.0

_cache = {}


def _build_l1(repeat=1):
    import concourse.bacc as bacc
    import concourse.bass as bass
    import concourse.mybir as mybir
    import concourse.tile as tile
    f32 = mybir.dt.float32
    Alu = mybir.AluOpType
    nc = bacc.Bacc("TRN2", target_bir_lowering=False, debug=False)
    xs = nc.dram_tensor("xs", [C, HW], f32, kind="ExternalInput")
    xbar = nc.dram_tensor("xbar", [1, HW], mybir.dt.bfloat16, kind="ExternalOutput")
    xb_t = xbar.ap().rearrange("one (p s) -> (one p) s", p=P)
    with tile.TileContext(nc) as tc:
        with tc.tile_pool(name="pl", bufs=2) as pool:
            for _ in range(repeat):
                t = pool.tile([P, C, HW // P], f32, tag="t", name="t")
                src = bass.AP(xs.ap().tensor, 0,
                              [[HW // P, P], [HW, C], [1, HW // P]])
                nc.sync.dma_start(t[:], src)
                nc.vector.tensor_tensor(t[:, 0:4, :], t[:, 0:4, :], t[:, 4:8, :],
                                        op=Alu.add)
                nc.vector.tensor_tensor(t[:, 0:2, :], t[:, 0:2, :], t[:, 2:4, :],
                                        op=Alu.add)
                nc.vector.tensor_tensor(t[:, 0:1, :], t[:, 0:1, :], t[:, 1:2, :],
                                        op=Alu.add)
                rb = pool.tile([P, HW // P], mybir.dt.bfloat16, tag="rb", name="rb")
                nc.vector.tensor_scalar_mul(rb[:], t[:, 0, :], 1.0 / C)
                nc.sync.dma_start(xb_t, rb[:])
    nc.compile()
    return nc


def _build_l2(repeat=1):
    import concourse.bacc as bacc
    import concourse.bass as bass
    import concourse.mybir as mybir
    import concourse.tile as tile
    f32 = mybir.dt.float32
    bf16 = mybir.dt.bfloat16
    i16 = mybir.dt.int16
    Alu = mybir.AluOpType

    nc = bacc.Bacc("TRN2", target_bir_lowering=False, debug=False)
    dimg = nc.dram_tensor("img", [P, NE * 2], bf16, kind="ExternalInput")
    tht = nc.dram_tensor("tht", [P, 8], f32, kind="ExternalInput")
    dwb = nc.dram_tensor("wb", [P, NI], mybir.dt.uint8, kind="ExternalInput")
    dhjd = nc.dram_tensor("hjd", [P, NR * NI], mybir.dt.uint8, kind="ExternalInput")
    dwci = nc.dram_tensor("wci", [P, NR * P], mybir.dt.uint8, kind="ExternalInput")
    dhjw = nc.dram_tensor("hjw", [P, NR * P], mybir.dt.uint8, kind="ExternalInput")
    daux = nc.dram_tensor("aux", [P, 4], f32, kind="ExternalInput")
    res = nc.dram_tensor("res", [B, HW], bf16, kind="ExternalOutput")

    with tile.TileContext(nc) as tc:
        with (
            tc.tile_pool(name="const", bufs=1) as cpool,
            tc.tile_pool(name="wk", bufs=1) as wp,
            tc.tile_pool(name="gt", bufs=1) as gp,
        ):
            img = cpool.tile([P, NE, 2], bf16, name="img")
            th = cpool.tile([P, 8], f32, name="th")
            Wb = cpool.tile([P, NI], mybir.dt.uint8, name="Wb")
            HJD = cpool.tile([P, NR * NI], mybir.dt.uint8, name="HJD")
            WCI = cpool.tile([P, NR * P], mybir.dt.uint8, name="WCI")
            HJW = cpool.tile([P, NR * P], mybir.dt.uint8, name="HJW")
            aux = cpool.tile([P, 4], f32, name="aux")
            nc.sync.dma_start(img[:], dimg.ap().rearrange("p (e l) -> p e l", l=2))
            nc.sync.dma_start(th[:], tht[:])
            nc.sync.dma_start(Wb[:], dwb[:])
            nc.sync.dma_start(HJD[:], dhjd[:])
            nc.sync.dma_start(WCI[:], dwci[:])
            nc.sync.dma_start(HJW[:], dhjw[:])
            nc.sync.dma_start(aux[:], daux[:])
            a_ = th[:, 0:1]
            b_ = th[:, 1:2]
            c0 = th[:, 2:3]
            d_ = th[:, 3:4]
            e_ = th[:, 4:5]
            f0 = th[:, 5:6]
            sel = aux[:, 1:2]
            omh = aux[:, 2:3]

            acc = cpool.tile([P, NR * NI], bf16, name="acc")

            for _ in range(repeat):
                # ===== wrapped-layout index pipeline, all rounds at once =====
                NW = NR * P  # 512
                TW = wp.tile([P, NW], f32, tag="TW", name="TW")
                UW = wp.tile([P, NW], f32, tag="UW", name="UW")
                FW = wp.tile([P, NW], bf16, tag="FW", name="FW")
                HW0 = wp.tile([P, NW], bf16, tag="HW0", name="HW0")
                RW0 = wp.tile([P, NW], bf16, tag="RW0", name="RW0")
                RW1 = wp.tile([P, NW], bf16, tag="RW1", name="RW1")
                QW = wp.tile([P, NW], bf16, tag="QW", name="QW")
                IDW = gp.tile([P, 4 * NW], i16, tag="IDW", name="IDW")
                nc.vector.tensor_scalar(TW[:], HJW[:], e_, f0,
                                        op0=Alu.mult, op1=Alu.add)
                nc.vector.scalar_tensor_tensor(TW[:], WCI[:], d_, TW[:],
                                               op0=Alu.mult, op1=Alu.add)
                nc.vector.tensor_scalar(TW[:], TW[:], -2.0, 257.5,
                                        op0=Alu.max, op1=Alu.min)
                nc.vector.tensor_scalar(TW[:], TW[:], 0.499, MAGIC,
                                        op0=Alu.subtract, op1=Alu.add)
                nc.vector.tensor_scalar(FW[:], TW[:], MAGIC, None,
                                        op0=Alu.subtract)
                nc.vector.tensor_scalar(HW0[:], FW[:], 2.0, -255.0,
                                        op0=Alu.mult, op1=Alu.add)
                nc.vector.tensor_scalar(HW0[:], HW0[:], 0.0, 1.0,
                                        op0=Alu.max, op1=Alu.min)
                nc.vector.scalar_tensor_tensor(RW0[:], HW0[:], -128.0, FW[:],
                                               op0=Alu.mult, op1=Alu.add)
                nc.vector.tensor_scalar(HW0[:], FW[:], 2.0, -253.0,
                                        op0=Alu.mult, op1=Alu.add)
                nc.vector.tensor_scalar(HW0[:], HW0[:], 0.0, 1.0,
                                        op0=Alu.max, op1=Alu.min)
                nc.vector.scalar_tensor_tensor(RW1[:], HW0[:], -128.0, FW[:],
                                               op0=Alu.mult, op1=Alu.add)
                nc.vector.tensor_scalar(RW1[:], RW1[:], 1.0, None, op0=Alu.add)
                nc.vector.tensor_scalar(UW[:], HJW[:], b_, c0,
                                        op0=Alu.mult, op1=Alu.add)
                nc.vector.scalar_tensor_tensor(UW[:], WCI[:], a_, UW[:],
                                               op0=Alu.mult, op1=Alu.add)
                nc.vector.tensor_scalar(UW[:], UW[:], -2.0, 257.5,
                                        op0=Alu.max, op1=Alu.min)
                nc.vector.tensor_scalar(UW[:], UW[:], 0.5, 0.499,
                                        op0=Alu.mult, op1=Alu.subtract)
                nc.vector.tensor_scalar(QW[:], UW[:], MAGIC, MAGIC,
                                        op0=Alu.add, op1=Alu.subtract)
                # idx layout: [call, R, c] so each (R, call) slice is contiguous
                for half in range(2):
                    RW = (RW0, RW1)[half]
                    for s in range(2):
                        ci_ = half * 2 + s
                        IDF = wp.tile([P, NW], f32, tag="IDF", name="IDF")
                        nc.vector.scalar_tensor_tensor(
                            IDF[:], RW[:], 130.0, QW[:], op0=Alu.mult, op1=Alu.add)
                        nc.vector.tensor_scalar(IDF[:], IDF[:], float(1 + s), 0.0,
                                                op0=Alu.add, op1=Alu.max)
                        nc.scalar.copy(IDW[:, ci_ * NW:(ci_ + 1) * NW], IDF[:])

                # ===== per-round weights + gathers =====
                for R in range(NR):
                    TA = wp.tile([P, NI], f32, tag="TA", name="TA")
                    VP = wp.tile([P, NI], f32, tag="VP", name="VP")
                    F = wp.tile([P, NI], bf16, tag="F", name="F")
                    FY = wp.tile([P, NI], bf16, tag="FY", name="FY")
                    HB0 = wp.tile([P, NI], bf16, tag="HB0", name="HB0")
                    HB1 = wp.tile([P, NI], bf16, tag="HB1", name="HB1")
                    WQ0 = wp.tile([P, NI], bf16, tag="WQ0", name="WQ0")
                    HJ = HJD[:, R * NI:(R + 1) * NI]
                    # y chain
                    nc.vector.tensor_scalar(TA[:], HJ, e_, f0,
                                            op0=Alu.mult, op1=Alu.add)
                    nc.vector.scalar_tensor_tensor(VP[:], Wb[:], d_, TA[:],
                                                   op0=Alu.mult, op1=Alu.add)
                    nc.vector.tensor_scalar(VP[:], VP[:], -2.0, 257.5,
                                            op0=Alu.max, op1=Alu.min)
                    nc.vector.tensor_scalar(TA[:], VP[:], 0.499, MAGIC,
                                            op0=Alu.subtract, op1=Alu.add)
                    nc.vector.tensor_scalar(F[:], TA[:], MAGIC, None,
                                            op0=Alu.subtract)
                    nc.vector.tensor_tensor(FY[:], VP[:], F[:], op=Alu.subtract)
                    nc.vector.tensor_scalar(HB0[:], F[:], 2.0, -255.0,
                                            op0=Alu.mult, op1=Alu.add)
                    nc.vector.tensor_scalar(HB0[:], HB0[:], 0.0, 1.0,
                                            op0=Alu.max, op1=Alu.min)
                    nc.vector.tensor_scalar(HB1[:], F[:], 2.0, -253.0,
                                            op0=Alu.mult, op1=Alu.add)
                    nc.vector.tensor_scalar(HB1[:], HB1[:], 0.0, 1.0,
                                            op0=Alu.max, op1=Alu.min)
                    nc.vector.tensor_scalar(HB0[:], HB0[:], sel, omh,
                                            op0=Alu.mult, op1=Alu.add)
                    nc.vector.tensor_scalar(HB1[:], HB1[:], sel, omh,
                                            op0=Alu.mult, op1=Alu.add)
                    nc.vector.tensor_scalar(WQ0[:], FY[:], -1.0, 1.0,
                                            op0=Alu.mult, op1=Alu.add)
                    nc.vector.tensor_tensor(WQ0[:], WQ0[:], HB0[:], op=Alu.mult)
                    nc.vector.tensor_tensor(FY[:], FY[:], HB1[:], op=Alu.mult)
                    # x chain (VP reused for xp)
                    TX = wp.tile([P, NI], bf16, tag="TX", name="TX")
                    WX0 = wp.tile([P, NI], bf16, tag="WX0", name="WX0")
                    WX1 = wp.tile([P, NI], bf16, tag="WX1", name="WX1")
                    WX2 = wp.tile([P, NI], bf16, tag="WX2", name="WX2")
                    Q = wp.tile([P, NI], bf16, tag="WX2", name="Qx")
                    nc.vector.tensor_scalar(TA[:], HJ, b_, c0,
                                            op0=Alu.mult, op1=Alu.add)
                    nc.vector.scalar_tensor_tensor(VP[:], Wb[:], a_, TA[:],
                                                   op0=Alu.mult, op1=Alu.add)
                    nc.vector.tensor_scalar(VP[:], VP[:], -2.0, 257.5,
                                            op0=Alu.max, op1=Alu.min)
                    nc.vector.tensor_scalar(TA[:], VP[:], 0.5, 0.499,
                                            op0=Alu.mult, op1=Alu.subtract)
                    nc.vector.tensor_scalar(Q[:], TA[:], MAGIC, MAGIC,
                                            op0=Alu.add, op1=Alu.subtract)
                    nc.vector.scalar_tensor_tensor(TX[:], Q[:], -2.0, VP[:],
                                                   op0=Alu.mult, op1=Alu.add)
                    nc.vector.tensor_scalar(WX0[:], TX[:], 1.0, None, op0=Alu.min)
                    nc.vector.tensor_scalar(WX0[:], WX0[:], -1.0, 1.0,
                                            op0=Alu.mult, op1=Alu.add)
                    nc.vector.tensor_scalar(WX1[:], TX[:], -1.0, 2.0,
                                            op0=Alu.mult, op1=Alu.add)
                    nc.vector.tensor_tensor(WX1[:], TX[:], WX1[:], op=Alu.min)
                    nc.vector.tensor_scalar(WX2[:], TX[:], 1.0, 0.0,
                                            op0=Alu.subtract, op1=Alu.max)
                    # gathers + combine
                    PS = wp.tile([P, NI], bf16, tag="PS", name="PS")
                    PT = wp.tile([P, NI], bf16, tag="PT", name="PT")
                    ACCR = PT  # in-place: PT dead once ACCR is formed
                    for half in range(2):
                        WQ = (WQ0, FY)[half]
                        for s in range(2):
                            ci_ = half * 2 + s
                            OUT = gp.tile([P, NI, 2], bf16, tag="OUT", name="OUT")
                            idxs = IDW[:, (ci_ * NR + R) * P:(ci_ * NR + R + 1) * P]
                            nc.gpsimd.ap_gather(
                                OUT[:], img[:], idxs,
                                channels=P, num_elems=NE, d=2, num_idxs=NI)
                            if s == 0:
                                WAL = wp.tile([P, NI, 2], bf16, tag="WAL",
                                              name="WAL")
                                nc.vector.tensor_tensor(WAL[:, :, 0], WQ[:],
                                                        WX0[:], op=Alu.mult)
                                nc.vector.tensor_tensor(WAL[:, :, 1], WQ[:],
                                                        WX1[:], op=Alu.mult)
                                nc.vector.tensor_tensor(OUT[:], OUT[:], WAL[:],
                                                        op=Alu.mult)
                                with nc.allow_low_precision("bf16 partials"):
                                    nc.vector.tensor_reduce(
                                        (PS, PT)[half][:], OUT[:],
                                        axis=mybir.AxisListType.X, op=Alu.add)
                            else:
                                WAL = wp.tile([P, NI, 2], bf16, tag="WAL",
                                              name="WAL")
                                nc.vector.tensor_tensor(WAL[:, :, 0], WQ[:],
                                                        WX2[:], op=Alu.mult)
                                nc.vector.tensor_tensor(WAL[:, :, 1], OUT[:, :, 0],
                                                        WAL[:, :, 0], op=Alu.mult)
                                with nc.allow_low_precision("bf16 partials"):
                                    if half == 0:
                                        nc.vector.tensor_tensor(
                                            PS[:], PS[:], WAL[:, :, 1], op=Alu.add)
                                    else:
                                        nc.vector.tensor_tensor(
                                            ACCR[:], PT[:], WAL[:, :, 1], op=Alu.add)
                    with nc.allow_low_precision("bf16 partials"):
                        nc.vector.tensor_tensor(ACCR[:], ACCR[:], PS[:],
                                                op=Alu.add)
                    # reorder list order j=c*16+k -> pixel order k*128+c
                    srcap = ACCR[:]
                    inap = bass.AP(srcap.tensor, srcap.offset,
                                   [srcap.ap[0], [1, 16], [16, 128]])
                    nc.vector.tensor_scalar(acc[:, R * NI:(R + 1) * NI], inap,
                                            0.0, None, op0=Alu.add)
                # ===== merge halves & write out =====
                HNI = NR * NI // 2
                for ch in range(2):
                    ST1 = wp.tile([64, HNI], bf16, tag="ST1", name="ST1")
                    ST2 = wp.tile([64, HNI], bf16, tag="ST2", name="ST2")
                    sl = slice(ch * HNI, (ch + 1) * HNI)
                    for g in range(8):
                        nc.sync.dma_start(ST1[g * 8:(g + 1) * 8, :],
                                          acc[g * 16 + 8:g * 16 + 16, sl])
                        nc.sync.dma_start(ST2[g * 8:(g + 1) * 8, :],
                                          acc[g * 16:g * 16 + 8, sl])
                    with nc.allow_low_precision("bf16 result"):
                        nc.vector.tensor_tensor(ST2[:], ST2[:], ST1[:], op=Alu.add)
                    for g in range(8):
                        dst = bass.AP(res.ap().tensor, g * NI + ch * 2 * 16384,
                                      [[HW, 8], [16384, 2], [1, NI]])
                        nc.sync.dma_start(dst, ST2[g * 8:(g + 1) * 8, :])
    nc.compile()
    return nc


def _consts():
    import ml_dtypes
    bf = ml_dtypes.bfloat16
    j = np.arange(NI, dtype=np.int64)
    wbv = ((j % 16) & 1) * 128 + (j >> 4)
    jdv = (j % 16) >> 1
    p = np.arange(P)
    g = p // 16
    jm = p % 16
    hbase = (np.arange(NR) * 64)[None, :, None] + (g * 8)[:, None, None]
    hjd = (hbase + jdv[None, None, :]).reshape(P, NR * NI).astype(np.uint8)
    wb = np.broadcast_to(wbv.astype(np.uint8), (P, NI))
    c = np.arange(P)
    wci4 = np.broadcast_to(
        ((128 * (jm & 1))[:, None] + c[None, :]).astype(np.uint8)[:, None, :],
        (P, NR, P)).reshape(P, NR * P)
    hjw4 = ((np.arange(NR) * 64)[None, :, None] + (g * 8 + (jm >> 1))[:, None, None]
            ).astype(np.uint8)
    hjw4 = np.broadcast_to(hjw4, (P, NR, P)).reshape(P, NR * P)
    aux = np.zeros((P, 4), np.float32)
    hfv = (jm // 8).astype(np.float32)
    aux[:, 0] = hfv
    aux[:, 1] = 2 * hfv - 1
    aux[:, 2] = 1 - hfv
    return (np.ascontiguousarray(wb), np.ascontiguousarray(hjd),
            np.ascontiguousarray(wci4), np.ascontiguousarray(hjw4),
            np.ascontiguousarray(aux))


def _build_img(xbb):
    import ml_dtypes
    bf = ml_dtypes.bfloat16
    img = np.zeros((P, NE, 2), bf)
    xb = np.asarray(xbb).reshape(8, 256, 256)
    for jm in range(16):
        b, hfv = jm % 8, jm // 8
        half = xb[b, 128 * hfv:128 * hfv + 128, :].reshape(128, 128, 2)
        one = np.zeros((NE, 2), bf)
        for rm in range(128):
            one[rm * RST + 1:rm * RST + 129] = half[rm]
        img[jm::16] = one[None, :, :]
    return np.ascontiguousarray(img.reshape(P, NE * 2))


def _theta_vec(th):
    a, bb = float(th[0, 0]), float(th[0, 1])
    c0 = 127.5 * (float(th[0, 2]) - a - bb + 1.0)
    d, e = float(th[1, 0]), float(th[1, 1])
    f0 = 127.5 * (float(th[1, 2]) - d - e + 1.0)
    vec = np.array([a, bb, c0, d, e, f0, 0.0, 0.0], np.float32)
    return np.ascontiguousarray(np.broadcast_to(vec, (P, 8)))


def _prep_in2(xbb, theta):
    wb, hjd, wci4, hjw4, aux = _consts()
    img = _build_img(xbb)
    in2 = []
    for o in range(B):
        in2.append({"img": img, "tht": _theta_vec(theta[o]), "wb": wb,
                    "hjd": hjd, "wci": wci4, "hjw": hjw4, "aux": aux})
    return in2


def kernel(x, theta):
    from concourse import bass_utils
    x = np.ascontiguousarray(x, dtype=np.float32)
    theta = np.ascontiguousarray(theta, dtype=np.float32)
    if "l1" not in _cache:
        _cache["l1"] = _build_l1()
        _cache["l2"] = _build_l2()
    l1, l2 = _cache["l1"], _cache["l2"]
    cores = list(range(8))
    in1 = [{"xs": np.ascontiguousarray(x[b].reshape(C, HW))} for b in range(B)]
    r1 = bass_utils.run_bass_kernel_spmd(l1, in1, core_ids=cores)
    xbb = np.ascontiguousarray(
        np.stack([r1.results[b]["xbar"].reshape(HW) for b in range(B)]))
    r2 = bass_utils.run_bass_kernel_spmd(l2, _prep_in2(xbb, theta), core_ids=cores)
    out = np.empty((B, B, H, W), np.float32)
    for o in range(B):
        out[:, o] = np.asarray(r2.results[o]["res"]).astype(np.float32).reshape(B, H, W)
    return out
